# revision 32
# baseline (speedup 1.0000x reference)
"""Trainium2 Bass kernel for a transformer decoder layer (self-attn + cross-attn + FFN).

Distribution over 8 NeuronCores:
  * self-attention: TP=8 over heads (2 heads/core) with BOTH batches
    concatenated on the token axis (4096 token-instances per core); the
    attention context is exchanged with a single 8-rank AllToAll (1MB) so each
    core ends up with all 1024 context features for its 512 tokens, then the
    full O-projection runs locally (no ReduceScatter).
  * cross-attention K/V: computed head-sharded per 4-core batch group from
    enc_out FIRST (before self-attention), with an interleaved head
    assignment (rank r owns heads {r, r+4, r+8, r+12}) so the AllGather can
    be split in two: AG1 (heads 0-7) completes early under self-attention,
    the AllToAlls run next, and AG2 (heads 8-15) finishes well before the
    second half of cross-attention needs it.  Cross-attention itself runs
    token-sharded with all 16 heads per core.
  * LayerNorms + FFN: token-sharded (512 tokens/core), no further comm.

All activations are feature-major ([D, seq]) on chip; matmuls are bf16 with
fp32 PSUM accumulation; softmax uses a constant exp-shift (exact for softmax)
with scores^T ([k, q]) layout.  The stationary [V | ones*64] trick makes each
attV matmul produce 64 broadcast copies of the softmax denominator, so
normalization is pure DVE (no PE broadcast matmuls).  LN statistics use an
all-ones [P,128] stationary so mean/var arrive pre-broadcast on 128
partitions.

Perf notes vs the first version:
  * scores for both heads of a pair land in one [128,1024] PSUM tile (2
    banks) so each softmax exp is a single ACT instruction over 1024
    columns — the ACT engine's 352-cycle fixed overhead per instruction
    was the single largest non-matmul cost.
  * softmax denominators use reciprocal_approx_fast (custom DVE, ~5x
    faster than InstReciprocal) read directly out of the attV PSUM.
  * LN rstd = exp(-0.5*ln(var+eps)): ln and exp live in the same ACT
    table set (natural_log_exp_and_others), so the kernel never pays the
    ~2.7us ACT table switch that AF.Sqrt forced.
  * PSUM is re-segmented per phase (projection pools vs attention pools)
    instead of statically split.
  * input DMAs are ordered by consumption: small weights first, then x/enc
    per 512-token block so the first projection matmul starts ~5us in.
"""

import sys

sys.path.insert(0, "/opt/trn_rl_repo")

import numpy as np
from ml_dtypes import bfloat16

import concourse.bass as bass
import concourse.mybir as mybir
import concourse.tile as tile
from concourse.tile_rust import add_dep_helper
from concourse import bacc
from concourse import bass_utils

F32 = mybir.dt.float32
BF16 = mybir.dt.bfloat16
ALU = mybir.AluOpType
AF = mybir.ActivationFunctionType

P = 128
EPS = 1e-5
EXP_SHIFT = -12.0  # exp(s + EXP_SHIFT): overflow headroom, exact for softmax
NEG = -1e9

SKIP = -2
FULL = -1

PIPE = 2  # scores-ahead-of-attV software pipeline depth
USE_FAST_RECIP = True


class Cfg:
    def __init__(self, S, D, H, FF, TP, B):
        self.S, self.D, self.H, self.FF, self.B = S, D, H, FF, B
        self.G = TP                  # cross-attn group size (per batch)
        self.DK = 64
        self.n_cores = TP * B        # 8
        self.TQ = B * S              # self-attn token instances
        self.HLs = H // self.n_cores # self local heads (2)
        self.MHs = self.HLs * self.DK
        self.NQ = self.TQ // 512     # self q blocks (8)
        self.NKs = self.TQ // P      # self k blocks (32)
        self.HLc = H // TP           # cross local heads for the AG (4)
        self.MHc = self.HLc * self.DK
        self.MOc = self.MHc // P
        self.NKc = S // P            # cross k blocks (16)
        self.SB = S // TP            # token chunk (512)
        self.DO = D // P
        self.FO = FF // P
        assert self.SB == 512 and self.MHs == P and self.HLs == 2


def build_program(cfg, self_cls, nsc, v_bias_zero=False, qk_bias_zero=False,
                  ln_identity=False, debug_stage=0):
    """Build + compile the SPMD program.

    self_cls: [NQ][NKs] entries SKIP / FULL / strip-index (identical on all
    cores).  nsc: number of cross strips (0 -> no cross mask work; else every
    cross block kb uses strip kb, strip data differs per core).
    qk_bias_zero: q/k/o projection biases are all zero -> drain PSUM through
    DVE casts instead of ACT identity.  ln_identity: all LN gammas are one
    and betas zero -> skip the LN affine pass.
    """
    S, D, DK, SB, TQ = cfg.S, cfg.D, cfg.DK, cfg.SB, cfg.TQ
    NQ, NKs, NKc, DO, FO, G, H = (cfg.NQ, cfg.NKs, cfg.NKc, cfg.DO, cfg.FO,
                                  cfg.G, cfg.H)
    HLs, MHs, HLc, MHc, MOc = cfg.HLs, cfg.MHs, cfg.HLc, cfg.MHc, cfg.MOc
    nss = max(1, 1 + max((v for row in self_cls for v in row if v >= 0),
                         default=-1))
    groups4 = [list(range(g * G, (g + 1) * G)) for g in range(cfg.B)]
    groups8 = [list(range(cfg.n_cores))]

    nc = bacc.Bacc("TRN2", target_bir_lowering=False, debug=False,
                   num_devices=cfg.n_cores)

    def din(name, shape, dt):
        return nc.dram_tensor(name, shape, dt, kind="ExternalInput").ap()

    xT = din("xT", [P, DO, TQ], BF16)
    xck = din("xck", [P, DO, SB], F32)
    encT = din("encT", [P, DO, S], BF16)
    wq1 = din("wq1", [P, DO, MHs], BF16)
    wk1 = din("wk1", [P, DO, MHs], BF16)
    wv1 = din("wv1", [P, DO, MHs], BF16)
    bq1 = din("bq1", [P, 1], F32)
    bk1 = din("bk1", [P, 1], F32)
    bv1 = din("bv1", [1, MHs], BF16)
    wo1p = din("wo1p", [P, DO, DO, P], BF16)
    bo1 = din("bo1", [P, DO], F32)
    wq2p = din("wq2p", [P, DO, DO, P], BF16)
    bq2 = din("bq2", [P, DO], F32)
    wk2 = din("wk2", [P, DO, MHc], BF16)
    bk2 = din("bk2", [P, MOc], F32)
    wv2 = din("wv2", [P, DO, MHc], BF16)
    bv2 = din("bv2", [1, MHc], BF16)
    wo2p = din("wo2p", [P, DO, DO, P], BF16)
    bo2 = din("bo2", [P, DO], F32)
    w1p = din("w1p", [P, FO, DO, P], BF16)
    b1 = din("b1", [P, FO], F32)
    w2p = din("w2p", [P, DO, FO, P], BF16)
    b2 = din("b2", [P, DO], F32)
    g1 = din("g1", [P, DO], F32)
    c1 = din("c1", [P, DO], F32)
    g2 = din("g2", [P, DO], F32)
    c2 = din("c2", [P, DO], F32)
    g3 = din("g3", [P, DO], F32)
    c3 = din("c3", [P, DO], F32)
    strS = din("strS", [P, nss, 1024], BF16)
    strC = din("strC", [P, max(nsc, 1), 2 * SB], BF16)
    y = nc.dram_tensor("y", [DO, P, SB], F32, kind="ExternalOutput").ap()

    with tile.TileContext(nc) as tc:
        res_cm = tc.tile_pool(name="res", bufs=1)
        res = res_cm.__enter__()
        dram_cm = tc.tile_pool(name="dram", bufs=1, space="DRAM")
        dram = dram_cm.__enter__()

        _mm_prev = [None]

        def mm_chain(inst):
            # Total-order the final readers of rotating "mm" PSUM tiles so the
            # scheduler cannot invert drain order against slot capacity.
            if _mm_prev[0] is not None:
                add_dep_helper(inst.ins, _mm_prev[0].ins, sync=True,
                               reason="mm drain order")
            _mm_prev[0] = inst

        psm_cm = tc.tile_pool(name="psm", bufs=2)
        psm = psm_cm.__enter__()
        pln_cm = tc.tile_pool(name="pln", bufs=1)
        pln = pln_cm.__enter__()
        patt_cm = tc.tile_pool(name="patt", bufs=3)
        patt = patt_cm.__enter__()
        phB_cm = tc.tile_pool(name="phB", bufs=1, side="right")
        phB = phB_cm.__enter__()

        # ---- resident small tensors (issued first: they are tiny) --------
        def load_res(ap_in, shape, dt, name):
            t = res.tile(shape, dt, name=name)
            nc.sync.dma_start(t[:], ap_in)
            return t

        bq1t = load_res(bq1, [P, 1], F32, "bq1t")
        bk1t = load_res(bk1, [P, 1], F32, "bk1t")
        bv1t = load_res(bv1, [1, MHs], BF16, "bv1t")
        bo1t = load_res(bo1, [P, DO], F32, "bo1t")
        bq2t = load_res(bq2, [P, DO], F32, "bq2t")
        bk2t = load_res(bk2, [P, MOc], F32, "bk2t")
        bv2t = load_res(bv2, [1, MHc], BF16, "bv2t")
        bo2t = load_res(bo2, [P, DO], F32, "bo2t")
        b1t = load_res(b1, [P, FO], F32, "b1t")
        b2t = load_res(b2, [P, DO], F32, "b2t")
        if not ln_identity:
            g1t = load_res(g1, [P, DO], F32, "g1t")
            c1t = load_res(c1, [P, DO], F32, "c1t")
            g2t = load_res(g2, [P, DO], F32, "g2t")
            c2t = load_res(c2, [P, DO], F32, "c2t")
            g3t = load_res(g3, [P, DO], F32, "g3t")
            c3t = load_res(c3, [P, DO], F32, "c3t")
        else:
            g1t = c1t = g2t = c2t = g3t = c3t = None
        strSt = load_res(strS, [P, nss, 1024], BF16, "strSt")
        strCt = load_res(strC, [P, max(nsc, 1), 2 * SB], BF16, "strCt") \
            if nsc > 0 else None
        onesb = res.tile([1, P], BF16, name="onesb")
        nc.vector.memset(onesb[:], 1.0)
        ones128 = res.tile([P, P], BF16, name="ones128")
        nc.vector.memset(ones128[:], 1.0)
        shiftc = res.tile([P, 1], F32, name="shiftc")
        nc.vector.memset(shiftc[:], EXP_SHIFT)
        epsc = res.tile([P, 1], F32, name="epsc")
        nc.vector.memset(epsc[:], EPS)

        # ---- input weights first, then x/enc per 512-token block ---------
        pio_e_cm = tc.tile_pool(name="pio_e", bufs=1)
        pio_e = pio_e_cm.__enter__()
        pio_x_cm = tc.tile_pool(name="pio_x", bufs=1)
        pio_x = pio_x_cm.__enter__()
        WQ1 = pio_x.tile([P, DO, MHs], BF16, name="WQ1")
        nc.sync.dma_start(WQ1[:], wq1)
        WK1 = pio_x.tile([P, DO, MHs], BF16, name="WK1")
        nc.sync.dma_start(WK1[:], wk1)
        WV1 = pio_x.tile([P, DO, MHs], BF16, name="WV1")
        nc.sync.dma_start(WV1[:], wv1)
        WK2 = pio_e.tile([P, DO, MHc], BF16, name="WK2")
        nc.sync.dma_start(WK2[:], wk2)
        WV2 = pio_e.tile([P, DO, MHc], BF16, name="WV2")
        nc.sync.dma_start(WV2[:], wv2)

        # ================= segment A: all QKV-style projections ===========
        ppA_cm = tc.tile_pool(name="ppA", bufs=3, space="PSUM")
        ppA = ppA_cm.__enter__()

        def qk_drain(dst, ps, bias_t):
            # PSUM -> SBUF bf16; bias add on ACT unless statically zero.
            if qk_bias_zero:
                mm_chain(nc.vector.tensor_copy(dst, ps))
            else:
                mm_chain(nc.scalar.activation(dst, ps, AF.Identity,
                                              bias=bias_t))

        def qk_proj(out, wt, bias_t, src, mo_n, seq):
            for m in range(mo_n):
                for n in range(seq // 512):
                    ps = ppA.tile([P, 512], F32, tag="mm", name="ps_qk")
                    for o in range(DO):
                        nc.tensor.matmul(
                            ps[:], wt[:, o, m * P:(m + 1) * P],
                            src[:, o, n * 512:(n + 1) * 512],
                            start=(o == 0), stop=(o == DO - 1))
                    qk_drain(out[:, m, n * 512:(n + 1) * 512], ps[:],
                             bias_t[:, m:m + 1] if bias_t is not None else None)

        def v_proj(out, wt, bias_row, src, hl, seq):
            # out: [P(tok), seq//P, hl, 2*DK]; cols DK: stay for the ones blk
            mh = hl * DK
            for ms in range(seq // P):
                ps = ppA.tile([P, 512], F32, tag="mm", name="ps_v")
                for o in range(DO):
                    nc.tensor.matmul(
                        ps[:, 0:mh], src[:, o, ms * P:(ms + 1) * P],
                        wt[:, o, :], start=(o == 0),
                        stop=(v_bias_zero and o == DO - 1))
                if not v_bias_zero:
                    nc.tensor.matmul(ps[:, 0:mh], onesb[0:1, :],
                                     bias_row[0:1, :], start=False, stop=True)
                mm_chain(nc.vector.tensor_copy(
                    out[:, ms, :, 0:DK],
                    ps[:, 0:mh].rearrange("p (h d) -> p h d", h=hl)))

        # ---- self QKV projections; KT1z[z] is head z's K zero-padded to
        # 128 partitions so scores run 128-contract (no row-mode switches) --
        QT1 = phB.tile([P, 1, TQ], BF16, name="QT1")
        KT1z = phB.tile([P, 2, TQ], BF16, name="KT1z")
        V1 = phB.tile([P, NKs, HLs, 2 * DK], BF16, name="V1")
        nc.gpsimd.memset(KT1z[DK:2 * DK, 0, :], 0.0)
        nc.gpsimd.memset(KT1z[0:DK, 1, :], 0.0)
        nc.gpsimd.memset(V1[:, :, :, DK:2 * DK], 1.0)
        # Q, K and V of each 512-token block are consumed back-to-back from
        # a rotating 3-deep block buffer: the PE keeps up with (not ahead
        # of) the per-block xT DMAs and only ~24KB/partition is staged.
        for n in range(TQ // 512):
            cols = slice(n * 512, (n + 1) * 512)
            XTb = pio_x.tile([P, DO, 512], BF16, tag="xtb", name="XTb",
                             bufs=3)
            nc.sync.dma_start(XTb[:], xT[:, :, cols])
            ps = ppA.tile([P, 512], F32, tag="mm", name="ps_q1")
            for o in range(DO):
                nc.tensor.matmul(ps[:], WQ1[:, o, :], XTb[:, o, :],
                                 start=(o == 0), stop=(o == DO - 1))
            qk_drain(QT1[:, 0, cols], ps[:], bq1t[:, 0:1])
            ps = ppA.tile([P, 512], F32, tag="mm", name="ps_k1")
            for o in range(DO):
                nc.tensor.matmul(ps[:], WK1[:, o, :], XTb[:, o, :],
                                 start=(o == 0), stop=(o == DO - 1))
            qk_drain(KT1z[0:DK, 0, cols], ps[0:DK], bk1t[0:DK, 0:1])
            qk_drain(KT1z[DK:2 * DK, 1, cols], ps[DK:2 * DK],
                     bk1t[DK:2 * DK, 0:1])
            for ms in range(4 * n, 4 * n + 4):
                ps = ppA.tile([P, 512], F32, tag="mm", name="ps_v")
                for o in range(DO):
                    nc.tensor.matmul(
                        ps[:, 0:MHs],
                        XTb[:, o, (ms % 4) * P:(ms % 4 + 1) * P],
                        WV1[:, o, :], start=(o == 0),
                        stop=(v_bias_zero and o == DO - 1))
                if not v_bias_zero:
                    nc.tensor.matmul(ps[:, 0:MHs], onesb[0:1, :],
                                     bv1t[0:1, :], start=False, stop=True)
                mm_chain(nc.vector.tensor_copy(
                    V1[:, ms, :, 0:DK],
                    ps[:, 0:MHs].rearrange("p (h d) -> p h d", h=HLs)))
        pio_x_cm.__exit__(None, None, None)

        # ---- cross K/V projections (head-interleaved; see module doc) ----
        # V is stored per AG-half as [slot, k, 2*DK] with the attv ones
        # block interleaved, so the AllGather ships attv-ready stationary
        # data and every DMA on the path is contiguous (2KB+ packets).
        KT2l = phB.tile([P, MOc, S], BF16, name="KT2l")
        V2lh = [phB.tile([P, 2, NKc, DK], BF16, name=f"V2lh{i}")
                for i in range(2)]
        for n in range(S // 512):
            cols = slice(n * 512, (n + 1) * 512)
            ENCb = pio_e.tile([P, DO, 512], BF16, tag="encb", name="ENCb",
                              bufs=3)
            nc.sync.dma_start(ENCb[:], encT[:, :, cols])
            for m in range(MOc):
                ps = ppA.tile([P, 512], F32, tag="mm", name="ps_k2")
                for o in range(DO):
                    nc.tensor.matmul(ps[:], WK2[:, o, m * P:(m + 1) * P],
                                     ENCb[:, o, :],
                                     start=(o == 0), stop=(o == DO - 1))
                qk_drain(KT2l[:, m, cols], ps[:], bk2t[:, m:m + 1])
            for ms in range(4 * n, 4 * n + 4):
                ps = ppA.tile([P, 512], F32, tag="mm", name="ps_v2")
                for o in range(DO):
                    nc.tensor.matmul(
                        ps[:, 0:MHc],
                        ENCb[:, o, (ms % 4) * P:(ms % 4 + 1) * P],
                        WV2[:, o, :], start=(o == 0),
                        stop=(v_bias_zero and o == DO - 1))
                if not v_bias_zero:
                    nc.tensor.matmul(ps[:, 0:MHc], onesb[0:1, :],
                                     bv2t[0:1, :], start=False, stop=True)
                for i in range(2):
                    mm_chain(nc.vector.tensor_copy(
                        V2lh[i][:, :, ms, :],
                        ps[:, i * P:(i + 1) * P].rearrange(
                            "p (h d) -> p h d", h=2)))
        pio_e_cm.__exit__(None, None, None)
        ppA_cm.__exit__(None, None, None)

        # each AG half: K mo-block (2 local heads) + matching V slots
        KH = S            # K half size per partition
        VH = 2 * NKc * DK # V half size per partition (compact)
        kv_loc = [dram.tile([P, KH + VH], BF16, name=f"kv_loc{i}")
                  for i in range(2)]
        kv_ag = [dram.tile([G, P, KH + VH], BF16, name=f"kv_ag{i}")
                 for i in range(2)]
        nc.sync.dma_start(kv_loc[0][:, 0:KH], KT2l[:, 0, :])
        nc.sync.dma_start(kv_loc[0][:, KH:KH + VH],
                          V2lh[0].rearrange("p s k d -> p (s k d)"))
        # AG1 now; the a2a's and then AG2 follow self-attention.  AG2's
        # input DMAs are also held back until then: the CC scheduler runs
        # whichever collective has ready inputs, and a ready AG2 would
        # block the latency-critical a2as for ~90us.
        nc.gpsimd.collective_compute(
            "AllGather", ALU.bypass, replica_groups=groups4,
            ins=[kv_loc[0].opt()], outs=[kv_ag[0].opt()])

        # cross-attn K/V staging allocated early: the ones-padding memsets
        # must land on the gpsimd queue BEFORE the a2a/AG2 collectives.
        # Cross attention runs in 4 head-quarters (4 heads each) to halve
        # the Va staging footprint; quarters 0,1 come from AG half 0.
        HQ = 4
        pva_cm = tc.tile_pool(name="pva", bufs=1)
        pva = pva_cm.__enter__()
        pkt_cm = tc.tile_pool(name="pkt", bufs=1)
        pkt = pkt_cm.__enter__()
        ktpz = [pkt.tile([P, 2, S], BF16, name=f"ktpz{i}") for i in range(2)]
        for i in range(2):
            nc.gpsimd.memset(ktpz[i][DK:2 * DK, 0, :], 0.0)
            nc.gpsimd.memset(ktpz[i][0:DK, 1, :], 0.0)
        VaB = [pva.tile([P, HQ, NKc, 2 * DK], BF16, name=f"VaB{i}")
               for i in range(2)]
        for i in range(2):
            nc.gpsimd.memset(VaB[i][:, :, :, DK:2 * DK], 1.0)


        # post-attention working set (DMAs run under attention) ------------
        pO1_cm = tc.tile_pool(name="pO1", bufs=1)
        pO1 = pO1_cm.__enter__()
        xckt = pO1.tile([P, DO, SB], F32, name="xckt")
        nc.sync.dma_start(xckt[:], xck)

        # ================= segment B: self attention ======================
        ppB_cm = tc.tile_pool(name="ppB", bufs=2, space="PSUM")
        ppB = ppB_cm.__enter__()

        def attention_pair(qt_m, KT, QT, vt, he, ho, qb, cls_row, strips,
                           Xatt, xm, nk):
            """One (head-pair, q-block).  Even head lives on SBUF partitions
            0-63, odd head on 64-127.  Vt[..., DK:2DK] is an all-ones block,
            so each attV matmul emits 64 broadcast copies of the softmax
            denominator on partitions 64-127; normalization is pure DVE.
            Both heads' scores land in one [128,1024] PSUM tile -> a single
            exp ACT instruction; scores run PIPE blocks ahead of attV."""
            allowed = [kb for kb in range(nk) if cls_row[kb] != SKIP]
            n = len(allowed)
            pa_e = ppB.tile([P, 512], F32, tag="pa", name="pa_e")
            pa_o = ppB.tile([P, 512], F32, tag="pa", name="pa_o")
            Es = {}

            def do_scores(j):
                kb = allowed[j]
                ps2 = ppB.tile([P, 1024], F32, tag="sc", name="ps_s", bufs=3)
                for half in (0, 1):
                    nc.tensor.matmul(
                        ps2[:, half * 512:half * 512 + 512],
                        KT[:, half, kb * P:(kb + 1) * P],
                        QT[:, qt_m, qb * 512:(qb + 1) * 512],
                        start=True, stop=True)
                if cls_row[kb] >= 0:
                    nc.vector.tensor_tensor(
                        ps2[:], ps2[:], strips[:, cls_row[kb], :], ALU.add)
                E = patt.tile([P, 1024], BF16, tag="E", name="E", bufs=3)
                mm_chain(nc.scalar.activation(E[:], ps2[:], AF.Exp,
                                              bias=shiftc[:, 0:1]))
                Es[j] = E

            def do_attv(j):
                kb = allowed[j]
                E = Es.pop(j)
                nc.tensor.matmul(pa_e[:], vt(kb, he), E[:, 0:512],
                                 start=(j == 0), stop=(j == n - 1))
                nc.tensor.matmul(pa_o[:], vt(kb, ho), E[:, 512:1024],
                                 start=(j == 0), stop=(j == n - 1))

            for j in range(n):
                do_scores(j)
                if j >= PIPE:
                    do_attv(j - PIPE)
            for j in range(max(0, n - PIPE), n):
                do_attv(j)

            # denominators: approx-reciprocal straight out of PSUM (fp32)
            rec = psm.tile([P, 512], F32, tag="rec", name="rec")
            if USE_FAST_RECIP:
                den = psm.tile([P, 512], F32, tag="den", name="den")
                nc.vector.tensor_copy(den[0:DK, :], pa_e[DK:2 * DK, :])
                nc.vector.tensor_copy(den[DK:2 * DK, :], pa_o[DK:2 * DK, :])
                nc.vector.reciprocal_approx_fast(rec[:, :], den[:, :])
            else:
                den = psm.tile([P, 512], BF16, tag="den", name="den")
                nc.vector.tensor_copy(den[0:DK, :], pa_e[DK:2 * DK, :])
                nc.vector.tensor_copy(den[DK:2 * DK, :], pa_o[DK:2 * DK, :])
                with nc.allow_low_precision(reason="softmax denom recip"):
                    nc.vector.reciprocal(rec[:, :], den[:, :])
            nc.vector.tensor_tensor(
                Xatt[0:DK, xm, :], pa_e[0:DK, :], rec[0:DK, :], ALU.mult)
            nc.vector.tensor_tensor(
                Xatt[DK:2 * DK, xm, :], pa_o[0:DK, :], rec[DK:2 * DK, :],
                ALU.mult)

        # ---- self attention + AllToAll context exchange ------------------
        HB = SB // 2
        a2a_inA = dram.tile([cfg.n_cores, P, HB], BF16, name="a2a_inA")
        a2a_inB = dram.tile([cfg.n_cores, P, HB], BF16, name="a2a_inB")

        a2a_in_insts = []

        def self_qb(qb):
            xst = pln.tile([P, 1, SB], BF16, tag="xst", name="xst", bufs=4)
            attention_pair(0, KT1z, QT1,
                           lambda kb, h: V1[:, kb, h, :], 0, 1, qb,
                           self_cls[qb], strSt, xst, 0, NKs)
            a2a_in_insts.append(nc.sync.dma_start(a2a_inA[qb],
                                                  xst[:, 0, 0:HB]))
            a2a_in_insts.append(nc.sync.dma_start(a2a_inB[qb],
                                                  xst[:, 0, HB:SB]))

        # interleave strip-heavy (small) and strip-light (large) q-blocks so
        # neither the DVE mask work nor the PE matmul work piles up.
        sz = lambda q: sum(1 for v in self_cls[q] if v != SKIP)
        asc = sorted(range(NQ), key=sz)
        qb_order = []
        for i in range(NQ // 2):
            qb_order += [asc[i], asc[NQ - 1 - i]]
        for qb in qb_order:
            self_qb(qb)
        ppB_cm.__exit__(None, None, None)

        a2a_outA = dram.tile([cfg.n_cores, P, HB], BF16, name="a2a_outA")
        a2a_outB = dram.tile([cfg.n_cores, P, HB], BF16, name="a2a_outB")
        nc.gpsimd.collective_compute(
            "AllToAll", ALU.bypass, replica_groups=groups8,
            ins=[a2a_inA.opt()], outs=[a2a_outA.opt()])
        nc.gpsimd.collective_compute(
            "AllToAll", ALU.bypass, replica_groups=groups8,
            ins=[a2a_inB.opt()], outs=[a2a_outB.opt()])
        # AG2's input write is chained behind the final a2a input so the
        # CC cannot start the (long) AG2 before the latency-critical a2as:
        # the CC scheduler runs whatever has ready inputs first.
        kvd1 = nc.sync.dma_start(kv_loc[1][:, 0:KH], KT2l[:, 1, :])
        kvd2 = nc.sync.dma_start(kv_loc[1][:, KH:KH + VH],
                                 V2lh[1].rearrange("p s k d -> p (s k d)"))
        for kvd in (kvd1, kvd2):
            add_dep_helper(kvd.ins, a2a_in_insts[-1].ins, sync=True,
                           reason="hold AG2 input behind a2a inputs")
        nc.gpsimd.collective_compute(
            "AllGather", ALU.bypass, replica_groups=groups4,
            ins=[kv_loc[1].opt()], outs=[kv_ag[1].opt()])
        phB_cm.__exit__(None, None, None)
        XA = pO1.tile([P, DO, SB], BF16, name="XA")
        nc.sync.dma_start(XA[:, :, 0:HB],
                          a2a_outA.rearrange("j p h -> p j h"))
        nc.sync.dma_start(XA[:, :, HB:SB],
                          a2a_outB.rearrange("j p h -> p j h"))

        # ================= segment C: O1 + LN1 + Q2 =======================
        ppC_cm = tc.tile_pool(name="ppC", bufs=2, space="PSUM")
        ppC = ppC_cm.__enter__()

        # ---- LN helpers (stats pre-broadcast via all-ones stationary;
        # per-m stat matmuls are emitted lag-one inside the producer loops
        # so the PE never drains between a projection and its LN) ----------
        def ln_stats_begin(pp):
            psA = pp.tile([P, 512], F32, tag="stat", name="psA", bufs=2)
            psB = pp.tile([P, 512], F32, tag="stat", name="psB", bufs=2)
            return psA, psB

        def ln_stat_m(st, S_sb, m, W):
            psA, psB = st
            sbf = pln.tile([P, W], BF16, tag="sbf", name="sbf", bufs=2)
            sq = pln.tile([P, W], BF16, tag="sq", name="sq", bufs=2)
            nc.vector.tensor_copy(sbf[:], S_sb[:, m])
            nc.vector.tensor_mul(sq[:], S_sb[:, m], S_sb[:, m])
            nc.tensor.matmul(psA[:, 0:W], ones128[:], sbf[:],
                             start=(m == 0), stop=(m == DO - 1))
            nc.tensor.matmul(psB[:, 0:W], ones128[:], sq[:],
                             start=(m == 0), stop=(m == DO - 1))

        def ln_finish(st, S_sb, gt, ct, out_f32, out_bf16, W,
                      out_cb=None):
            psA, psB = st
            mu = psm.tile([P, 512], F32, tag="stat", name="mu", bufs=6)
            nc.vector.tensor_scalar_mul(mu[:, 0:W], psA[:, 0:W], 1.0 / D)
            e2 = psm.tile([P, 512], F32, tag="stat", name="e2", bufs=6)
            nc.vector.tensor_scalar_mul(e2[:, 0:W], psB[:, 0:W], 1.0 / D)
            var = psm.tile([P, 512], F32, tag="stat", name="var", bufs=6)
            nc.vector.tensor_mul(var[:, 0:W], mu[:, 0:W], mu[:, 0:W])
            nc.vector.tensor_sub(var[:, 0:W], e2[:, 0:W], var[:, 0:W])
            # rstd = exp(-0.5 * ln(var + eps)) -- stays in the exp table set
            lnv = psm.tile([P, 512], F32, tag="stat", name="lnv", bufs=6)
            nc.scalar.activation(lnv[:, 0:W], var[:, 0:W], AF.Ln,
                                 bias=epsc[:, 0:1])
            rstd = psm.tile([P, 512], F32, tag="stat", name="rstd", bufs=6)
            nc.scalar.activation(rstd[:, 0:W], lnv[:, 0:W], AF.Exp,
                                 scale=-0.5)
            mr = psm.tile([P, 512], F32, tag="stat", name="mr", bufs=6)
            nc.vector.tensor_mul(mr[:, 0:W], mu[:, 0:W], rstd[:, 0:W])
            for m in range(DO):
                if ln_identity:
                    t2 = pln.tile([P, W], F32, tag="t2", name="t2", bufs=2)
                    nc.vector.tensor_mul(t2[:], S_sb[:, m], rstd[:, 0:W])
                    nc.vector.tensor_sub(out_f32[:, m], t2[:], mr[:, 0:W])
                else:
                    t2 = pln.tile([P, W], F32, tag="t2", name="t2", bufs=2)
                    nc.vector.tensor_mul(t2[:], S_sb[:, m], rstd[:, 0:W])
                    nc.vector.tensor_sub(t2[:], t2[:], mr[:, 0:W])
                    nc.scalar.activation(out_f32[:, m], t2[:], AF.Identity,
                                         bias=ct[:, m:m + 1],
                                         scale=gt[:, m:m + 1])
                if out_bf16 is not None:
                    nc.vector.tensor_copy(out_bf16[:, m], out_f32[:, m])
                if out_cb is not None:
                    out_cb(m)

        pw_cm = tc.tile_pool(name="pw", bufs=4, side="right")
        pw = pw_cm.__enter__()
        pC_cm = tc.tile_pool(name="pC", bufs=1, side="right")
        pC = pC_cm.__enter__()

        # ---- O1 projection (full D) + residual + LN1 ---------------------
        S1 = pO1.tile([P, DO, SB], F32, name="S1")
        st1 = ln_stats_begin(ppC)
        for m in range(DO):
            wo1t = pw.tile([P, DO, P], BF16, tag="wsm", name="wo1t")
            nc.sync.dma_start(wo1t[:], wo1p[:, m])
            ps = ppC.tile([P, 512], F32, tag="mm", name="ps_o1", bufs=3)
            for j in range(DO):
                nc.tensor.matmul(
                    ps[:, 0:SB], wo1t[:, j, :],
                    XA[:, j, :], start=(j == 0), stop=(j == DO - 1))
            mm_chain(nc.vector.scalar_tensor_tensor(
                S1[:, m, :], ps[:, 0:SB], bo1t[:, m:m + 1],
                xckt[:, m, :], ALU.add, ALU.add))
            if m > 0:
                ln_stat_m(st1, S1, m - 1, SB)
        ln_stat_m(st1, S1, DO - 1, SB)
        X2f = pC.tile([P, DO, SB], F32, name="X2f")
        X2b = pC.tile([P, DO, SB], BF16, name="X2b")
        ln_finish(st1, S1, g1t, c1t, X2f, X2b, SB)
        if debug_stage == 1:
            for m in range(DO):
                nc.sync.dma_start(y[m], X2f[:, m])
        pO1_cm.__exit__(None, None, None)

        # ---- cross-attn Q projection (token-sharded, all heads) ----------
        Q2T = pC.tile([P, DO, SB], BF16, name="Q2T")
        for m in range(DO):
            wq2t = pw.tile([P, DO, P], BF16, tag="wsm", name="wq2t")
            nc.sync.dma_start(wq2t[:], wq2p[:, m])
            ps = ppC.tile([P, 512], F32, tag="mm", name="ps_q2", bufs=3)
            for o in range(DO):
                nc.tensor.matmul(ps[:, 0:SB], wq2t[:, o, :], X2b[:, o, :],
                                 start=(o == 0), stop=(o == DO - 1))
            qk_drain(Q2T[:, m, :], ps[:, 0:SB], bq2t[:, m:m + 1])
        ppC_cm.__exit__(None, None, None)

        # ================= segment D: cross attention =====================
        ppD_cm = tc.tile_pool(name="ppD", bufs=2, space="PSUM")
        ppD = ppD_cm.__enter__()
        # reuse attention_pair's pool variable
        ppB = ppD

        # head h lives on AG rank h%G, local slot (h//G)%2 of half h//(2*G)
        Xatt2 = pC.tile([P, DO, SB], BF16, name="Xatt2")
        cross_cls = [kb if nsc > 0 else FULL for kb in range(NKc)]
        VSL = NKc * DK  # one slot's V elements per partition
        for q in range(H // HQ):
            Va = VaB[q % 2]
            half = q // 2
            h0 = q * HQ
            for h in range(h0, h0 + HQ):
                r, s = h % G, (h // G) % 2
                Vc = pva.tile([P, NKc * DK], BF16, tag="vac", name="Vc",
                              bufs=4)
                nc.sync.dma_start(
                    Vc[:],
                    kv_ag[half][r, :, KH + s * VSL:KH + (s + 1) * VSL])
                nc.vector.tensor_copy(
                    Va[:, h - h0, :, 0:DK],
                    Vc[:].rearrange("p (k d) -> p k d", k=NKc))
            for hp in range(h0 // 2, (h0 + HQ) // 2):
                kz = ktpz[hp % 2]
                for z, h in ((0, 2 * hp), (1, 2 * hp + 1)):
                    r, s = h % G, (h // G) % 2
                    nc.sync.dma_start(
                        kz[z * DK:(z + 1) * DK, z, :],
                        kv_ag[half][r, s * DK:(s + 1) * DK, 0:KH])
                attention_pair(hp, kz, Q2T,
                               lambda kb, h: Va[:, h, kb, :],
                               2 * hp - h0, 2 * hp + 1 - h0, 0, cross_cls,
                               strCt, Xatt2, hp, NKc)
        pkt_cm.__exit__(None, None, None)
        pva_cm.__exit__(None, None, None)
        ppD_cm.__exit__(None, None, None)

        # ================= segment E: O2 + LN2 + FFN + LN3 ================
        ppE_cm = tc.tile_pool(name="ppE", bufs=2, space="PSUM")
        ppE = ppE_cm.__enter__()
        pff_cm = tc.tile_pool(name="pff", bufs=1)
        pff = pff_cm.__enter__()
        FH = FO // 2
        # prefetch the first fc1 weight half while cross O-proj runs; the
        # second half streams through pw during the first half's matmuls
        W1h0 = pff.tile([P, FH, DO, P], BF16, name="W1h0")
        for mf in range(FH):
            nc.sync.dma_start(W1h0[:, mf], w1p[:, mf])

        # ---- cross O-projection + residual (in-place over X2f) + LN2 ----
        st2 = ln_stats_begin(ppE)
        for m in range(DO):
            wo2t = pw.tile([P, DO, P], BF16, tag="wsm", name="wo2t")
            nc.sync.dma_start(wo2t[:], wo2p[:, m])
            ps = ppE.tile([P, 512], F32, tag="mm", name="ps_o2", bufs=3)
            for o in range(DO):
                nc.tensor.matmul(ps[:, 0:SB], wo2t[:, o, :], Xatt2[:, o, :],
                                 start=(o == 0), stop=(o == DO - 1))
            mm_chain(nc.vector.scalar_tensor_tensor(
                X2f[:, m], ps[:, 0:SB], bo2t[:, m:m + 1], X2f[:, m],
                ALU.add, ALU.add))
            if m > 0:
                ln_stat_m(st2, X2f, m - 1, SB)
        ln_stat_m(st2, X2f, DO - 1, SB)
        X4f = pff.tile([P, DO, SB], F32, name="X4f")
        X4b = pff.tile([P, DO, SB], BF16, name="X4b")
        ln_finish(st2, X2f, g2t, c2t, X4f, X4b, SB)
        if debug_stage == 2:
            for m in range(DO):
                nc.sync.dma_start(y[m], X4f[:, m])
        pC_cm.__exit__(None, None, None)

        # ---- FFN (two FO-halves; accumulate into X4f in place) ----------
        st3 = ln_stats_begin(ppE)
        for half in range(2):
            f0 = half * FH
            Ht = pff.tile([P, FH, SB], BF16, tag="Ht", name="Ht", bufs=2)
            for mf in range(FH):
                if half == 0:
                    w1t = W1h0[:, mf]
                else:
                    w1t = pw.tile([P, DO, P], BF16, tag="wsm", name="w1t")
                    nc.sync.dma_start(w1t[:], w1p[:, f0 + mf])
                ps = ppE.tile([P, 512], F32, tag="mm", name="ps_f1", bufs=3)
                for o in range(DO):
                    nc.tensor.matmul(ps[:, 0:SB], w1t[:, o, :],
                                     X4b[:, o, :],
                                     start=(o == 0), stop=(o == DO - 1))
                mm_chain(nc.scalar.activation(
                    Ht[:, mf, :], ps[:, 0:SB], AF.Relu,
                    bias=b1t[:, f0 + mf:f0 + mf + 1]))
            for m in range(DO):
                w2t = pw.tile([P, FH, P], BF16, tag="w2", name="w2t", bufs=3)
                nc.sync.dma_start(w2t[:], w2p[:, m, f0:f0 + FH, :])
                ps = ppE.tile([P, 512], F32, tag="mm", name="ps_f2", bufs=3)
                for of in range(FH):
                    nc.tensor.matmul(ps[:, 0:SB], w2t[:, of, :], Ht[:, of, :],
                                     start=(of == 0), stop=(of == FH - 1))
                if half == 0:
                    mm_chain(nc.vector.scalar_tensor_tensor(
                        X4f[:, m], ps[:, 0:SB], b2t[:, m:m + 1], X4f[:, m],
                        ALU.add, ALU.add))
                else:
                    mm_chain(nc.vector.tensor_add(
                        X4f[:, m], X4f[:, m], ps[:, 0:SB]))
                    if m > 0:
                        ln_stat_m(st3, X4f, m - 1, SB)
        ln_stat_m(st3, X4f, DO - 1, SB)
        ln_finish(st3, X4f, g3t, c3t, X4f, None, SB,
                  out_cb=(lambda m: nc.sync.dma_start(y[m], X4f[:, m]))
                  if debug_stage == 0 else None)

        pff_cm.__exit__(None, None, None)
        ppE_cm.__exit__(None, None, None)
        pw_cm.__exit__(None, None, None)
        patt_cm.__exit__(None, None, None)
        pln_cm.__exit__(None, None, None)
        pC_cm.__exit__(None, None, None)
        psm_cm.__exit__(None, None, None)
        dram_cm.__exit__(None, None, None)
        res_cm.__exit__(None, None, None)

    nc.compile()
    return nc


# ---------------------------------------------------------------------------
# host side
# ---------------------------------------------------------------------------

def _pack_ko(a):
    """[K, F] -> [128, K//128, F] (contract dim on partitions)."""
    K, F = a.shape
    return np.ascontiguousarray(a.reshape(K // P, P, F).transpose(1, 0, 2))


def _pack_vec(v, n):
    return np.ascontiguousarray(np.asarray(v, np.float32).reshape(n, P).T)


def classify_self(mask, NQ, NK):
    """mask [S, S] bool (q, k). Returns cls [NQ][NK] and strips [128, nss, 512]."""
    cls = [[FULL] * NK for _ in range(NQ)]
    strips = []
    keys = {}
    for qb in range(NQ):
        for kb in range(NK):
            blk = mask[qb * 512:(qb + 1) * 512, kb * P:(kb + 1) * P]
            if blk.all():
                cls[qb][kb] = FULL
            elif not blk.any():
                cls[qb][kb] = SKIP
            else:
                key = blk.tobytes()
                if key not in keys:
                    keys[key] = len(strips)
                    strips.append(np.where(blk.T, np.float32(0),
                                           np.float32(NEG)))
                cls[qb][kb] = keys[key]
    if strips:
        arr = np.stack(strips, 0).transpose(1, 0, 2)
    else:
        arr = np.zeros((P, 1, 512), np.float32)
    arr = np.concatenate([arr, arr], axis=2)  # same strip for both heads
    return cls, np.ascontiguousarray(arr).astype(bfloat16)


_CACHE = {}
DEBUG_STAGE = 0


def kernel(**inputs):
    cfg = Cfg(S=2048, D=1024, H=16, FF=4096, TP=4, B=2)
    return _run(cfg, inputs)


def _run(cfg, inputs, trace=False):
    S, D, G, B, SB, DO = cfg.S, cfg.D, cfg.G, cfg.B, cfg.SB, cfg.DO
    MHs, MHc, MOc, NQ, NKs, NKc = (cfg.MHs, cfg.MHc, cfg.MOc, cfg.NQ,
                                   cfg.NKs, cfg.NKc)
    f32 = np.float32
    bf = bfloat16
    tgt = np.asarray(inputs["tgt_mask"])[0, 0] != 0
    src = np.asarray(inputs["src_mask"])[0, 0] != 0

    # per-batch causal classification, composed block-diagonally over B
    clsb, strS = classify_self(tgt, S // 512, S // P)
    nqb, nkb = S // 512, S // P
    self_cls = [[SKIP] * NKs for _ in range(NQ)]
    for qb in range(NQ):
        for kb in range(NKs):
            if qb // nqb == kb // nkb:
                self_cls[qb][kb] = clsb[qb % nqb][kb % nkb]
    nsc = 0 if src.all() else NKc

    v_bias_zero = (not np.asarray(inputs["m1_bv"]).any()) and \
        (not np.asarray(inputs["m2_bv"]).any())
    qk_bias_zero = not any(np.asarray(inputs[k]).any() for k in
                           ("m1_bq", "m1_bk", "m2_bq", "m2_bk"))
    ln_identity = all(
        (np.asarray(inputs[g]) == 1).all() and
        (not np.asarray(inputs[c]).any())
        for g, c in (("ln1_g", "ln1_b"), ("ln2_g", "ln2_b"),
                     ("ln3_g", "ln3_b")))
    key = (cfg.S, cfg.D, cfg.H, cfg.FF, cfg.G, cfg.B,
           tuple(map(tuple, self_cls)), nsc, v_bias_zero, qk_bias_zero,
           ln_identity, DEBUG_STAGE, USE_FAST_RECIP)
    if key not in _CACHE:
        _CACHE[key] = build_program(cfg, self_cls, nsc, v_bias_zero,
                                    qk_bias_zero, ln_identity, DEBUG_STAGE)
    nc = _CACHE[key]

    x = np.asarray(inputs["x"], f32)
    enc = np.asarray(inputs["enc_out"], f32)
    w1 = np.asarray(inputs["ff_w1"], f32)
    w2 = np.asarray(inputs["ff_w2"], f32)
    wq2 = np.asarray(inputs["m2_wq"], f32)
    wo2 = np.asarray(inputs["m2_wo"], f32)

    # xT: both batches concatenated on the token axis (batch-major)
    xT_full = np.concatenate([x[0], x[1]], axis=0).T  # [D, TQ]
    xT_pack = np.ascontiguousarray(
        xT_full.reshape(DO, P, cfg.TQ).transpose(1, 0, 2)).astype(bf)

    shared = {
        "xT": xT_pack,
        "wo1p": np.ascontiguousarray(
            np.asarray(inputs["m1_wo"], f32).reshape(
                DO, P, DO, P).transpose(1, 2, 0, 3)).astype(bf),
        "wq2p": np.ascontiguousarray(
            wq2.reshape(DO, P, DO, P).transpose(1, 2, 0, 3)).astype(bf),
        "bq2": _pack_vec(inputs["m2_bq"], DO),
        "wo2p": np.ascontiguousarray(
            wo2.reshape(DO, P, DO, P).transpose(1, 2, 0, 3)).astype(bf),
        "bo2": _pack_vec(inputs["m2_bo"], DO),
        "bo1": _pack_vec(inputs["m1_bo"], DO),
        "w1p": np.ascontiguousarray(
            w1.reshape(DO, P, cfg.FO, P).transpose(1, 2, 0, 3)).astype(bf),
        "b1": _pack_vec(inputs["ff_b1"], cfg.FO),
        "w2p": np.ascontiguousarray(
            w2.reshape(cfg.FO, P, DO, P).transpose(1, 2, 0, 3)).astype(bf),
        "b2": _pack_vec(inputs["ff_b2"], DO),
        "g1": _pack_vec(inputs["ln1_g"], DO),
        "c1": _pack_vec(inputs["ln1_b"], DO),
        "g2": _pack_vec(inputs["ln2_g"], DO),
        "c2": _pack_vec(inputs["ln2_b"], DO),
        "g3": _pack_vec(inputs["ln3_g"], DO),
        "c3": _pack_vec(inputs["ln3_b"], DO),
        "strS": strS,
    }

    in_maps = []
    for c in range(cfg.n_cores):
        b, r = divmod(c, G)
        xTb = np.ascontiguousarray(x[b].T)
        encTb = np.ascontiguousarray(enc[b].T)
        m = dict(shared)
        m["xck"] = np.ascontiguousarray(
            xTb[:, r * SB:(r + 1) * SB].reshape(DO, P, SB).transpose(1, 0, 2))
        m["encT"] = np.ascontiguousarray(
            encTb.reshape(DO, P, S).transpose(1, 0, 2)).astype(bf)
        # self-attn: 2 heads per core (TP=8 over heads)
        hs = slice(c * MHs, (c + 1) * MHs)
        m["wq1"] = _pack_ko(np.asarray(inputs["m1_wq"], f32)[:, hs]).astype(bf)
        m["wk1"] = _pack_ko(np.asarray(inputs["m1_wk"], f32)[:, hs]).astype(bf)
        m["wv1"] = _pack_ko(np.asarray(inputs["m1_wv"], f32)[:, hs]).astype(bf)
        m["bq1"] = _pack_vec(np.asarray(inputs["m1_bq"], f32)[hs], 1)
        m["bk1"] = _pack_vec(np.asarray(inputs["m1_bk"], f32)[hs], 1)
        m["bv1"] = np.asarray(inputs["m1_bv"], f32)[hs].reshape(1, MHs).astype(bf)
        # cross-attn K/V: interleaved heads {r, r+G, r+2G, r+3G} per rank,
        # packed in slot order so AG half i carries slots 2i, 2i+1
        DKc = cfg.DK
        heads_r = [r + G * s for s in range(cfg.HLc)]
        hc_cols = np.concatenate(
            [np.arange(h * DKc, (h + 1) * DKc) for h in heads_r])
        m["wk2"] = _pack_ko(
            np.asarray(inputs["m2_wk"], f32)[:, hc_cols]).astype(bf)
        m["wv2"] = _pack_ko(
            np.asarray(inputs["m2_wv"], f32)[:, hc_cols]).astype(bf)
        m["bk2"] = _pack_vec(np.asarray(inputs["m2_bk"], f32)[hc_cols], MOc)
        m["bv2"] = np.asarray(inputs["m2_bv"], f32)[hc_cols].reshape(
            1, MHc).astype(bf)
        if nsc > 0:
            blks = []
            for kb in range(NKc):
                blk = src[r * SB:(r + 1) * SB, kb * P:(kb + 1) * P]
                blks.append(np.where(blk.T, f32(0), f32(NEG)))
            arrc = np.stack(blks, 0).transpose(1, 0, 2)
            arrc = np.concatenate([arrc, arrc], axis=2)
            m["strC"] = np.ascontiguousarray(arrc).astype(bf)
        else:
            m["strC"] = np.zeros((P, 1, 2 * SB), bf)
        in_maps.append(m)

    res = bass_utils.run_bass_kernel_spmd(
        nc, in_maps, core_ids=list(range(cfg.n_cores)), trace=trace)

    out = np.empty((B, S, D), f32)
    for c in range(cfg.n_cores):
        b, r = divmod(c, G)
        yv = res.results[c]["y"]
        out[b, r * SB:(r + 1) * SB, :] = yv.transpose(2, 0, 1).reshape(SB, D)
    if trace:
        return out, res
    return out


# revision 34
# speedup vs baseline: 1.0610x; 1.0610x over previous
"""Trainium2 Bass kernel for a transformer decoder layer (self-attn + cross-attn + FFN).

Distribution over 8 NeuronCores:
  * self-attention: TP=8 over heads (2 heads/core) with BOTH batches
    concatenated on the token axis (4096 token-instances per core); the
    attention context is exchanged with a single 8-rank AllToAll (1MB) so each
    core ends up with all 1024 context features for its 512 tokens, then the
    full O-projection runs locally (no ReduceScatter).
  * cross-attention K/V: computed head-sharded per 4-core batch group from
    enc_out FIRST (before self-attention), with an interleaved head
    assignment (rank r owns heads {r, r+4, r+8, r+12}) so the AllGather can
    be split in two: AG1 (heads 0-7) completes early under self-attention,
    the AllToAlls run next, and AG2 (heads 8-15) finishes well before the
    second half of cross-attention needs it.  Cross-attention itself runs
    token-sharded with all 16 heads per core.
  * LayerNorms + FFN: token-sharded (512 tokens/core), no further comm.

All activations are feature-major ([D, seq]) on chip; matmuls are bf16 with
fp32 PSUM accumulation; softmax uses a constant exp-shift (exact for softmax)
with scores^T ([k, q]) layout.  The stationary [V | ones*64] trick makes each
attV matmul produce 64 broadcast copies of the softmax denominator, so
normalization is pure DVE (no PE broadcast matmuls).  LN statistics use an
all-ones [P,128] stationary so mean/var arrive pre-broadcast on 128
partitions.

Perf notes vs the first version:
  * scores for both heads of a pair land in one [128,1024] PSUM tile (2
    banks) so each softmax exp is a single ACT instruction over 1024
    columns — the ACT engine's 352-cycle fixed overhead per instruction
    was the single largest non-matmul cost.
  * softmax denominators use reciprocal_approx_fast (custom DVE, ~5x
    faster than InstReciprocal) read directly out of the attV PSUM.
  * LN rstd = exp(-0.5*ln(var+eps)): ln and exp live in the same ACT
    table set (natural_log_exp_and_others), so the kernel never pays the
    ~2.7us ACT table switch that AF.Sqrt forced.
  * PSUM is re-segmented per phase (projection pools vs attention pools)
    instead of statically split.
  * input DMAs are ordered by consumption: small weights first, then x/enc
    per 512-token block so the first projection matmul starts ~5us in.
"""

import sys

sys.path.insert(0, "/opt/trn_rl_repo")

import numpy as np
from ml_dtypes import bfloat16

import concourse.bass as bass
import concourse.mybir as mybir
import concourse.tile as tile
from concourse.tile_rust import add_dep_helper
from concourse import bacc
from concourse import bass_utils

F32 = mybir.dt.float32
BF16 = mybir.dt.bfloat16
ALU = mybir.AluOpType
AF = mybir.ActivationFunctionType

P = 128
EPS = 1e-5
EXP_SHIFT = -12.0  # exp(s + EXP_SHIFT): overflow headroom, exact for softmax
NEG = -1e9

SKIP = -2
FULL = -1

PIPE = 3  # scores-ahead-of-attV software pipeline depth
USE_FAST_RECIP = True


class Cfg:
    def __init__(self, S, D, H, FF, TP, B):
        self.S, self.D, self.H, self.FF, self.B = S, D, H, FF, B
        self.G = TP                  # cross-attn group size (per batch)
        self.DK = 64
        self.n_cores = TP * B        # 8
        self.TQ = B * S              # self-attn token instances
        self.HLs = H // self.n_cores # self local heads (2)
        self.MHs = self.HLs * self.DK
        self.NQ = self.TQ // 512     # self q blocks (8)
        self.NKs = self.TQ // P      # self k blocks (32)
        self.HLc = H // TP           # cross local heads for the AG (4)
        self.MHc = self.HLc * self.DK
        self.MOc = self.MHc // P
        self.NKc = S // P            # cross k blocks (16)
        self.SB = S // TP            # token chunk (512)
        self.DO = D // P
        self.FO = FF // P
        assert self.SB == 512 and self.MHs == P and self.HLs == 2


def build_program(cfg, self_cls, nsc, v_bias_zero=False, qk_bias_zero=False,
                  ln_identity=False, debug_stage=0):
    """Build + compile the SPMD program.

    self_cls: [NQ][NKs] entries SKIP / FULL / strip-index (identical on all
    cores).  nsc: number of cross strips (0 -> no cross mask work; else every
    cross block kb uses strip kb, strip data differs per core).
    qk_bias_zero: q/k/o projection biases are all zero -> drain PSUM through
    DVE casts instead of ACT identity.  ln_identity: all LN gammas are one
    and betas zero -> skip the LN affine pass.
    """
    S, D, DK, SB, TQ = cfg.S, cfg.D, cfg.DK, cfg.SB, cfg.TQ
    NQ, NKs, NKc, DO, FO, G, H = (cfg.NQ, cfg.NKs, cfg.NKc, cfg.DO, cfg.FO,
                                  cfg.G, cfg.H)
    HLs, MHs, HLc, MHc, MOc = cfg.HLs, cfg.MHs, cfg.HLc, cfg.MHc, cfg.MOc
    nss = max(1, 1 + max((v for row in self_cls for v in row if v >= 0),
                         default=-1))
    groups4 = [list(range(g * G, (g + 1) * G)) for g in range(cfg.B)]
    groups8 = [list(range(cfg.n_cores))]

    nc = bacc.Bacc("TRN2", target_bir_lowering=False, debug=False,
                   num_devices=cfg.n_cores)

    def din(name, shape, dt):
        return nc.dram_tensor(name, shape, dt, kind="ExternalInput").ap()

    xT = din("xT", [P, DO, TQ], BF16)
    xck = din("xck", [P, DO, SB], F32)
    encT = din("encT", [P, DO, S], BF16)
    wq1 = din("wq1", [P, DO, MHs], BF16)
    wk1 = din("wk1", [P, DO, MHs], BF16)
    wv1 = din("wv1", [P, DO, MHs], BF16)
    bq1 = din("bq1", [P, 1], F32)
    bk1 = din("bk1", [P, 1], F32)
    bv1 = din("bv1", [1, MHs], BF16)
    wo1p = din("wo1p", [P, DO, DO, P], BF16)
    bo1 = din("bo1", [P, DO], F32)
    wq2p = din("wq2p", [P, DO, DO, P], BF16)
    bq2 = din("bq2", [P, DO], F32)
    wk2 = din("wk2", [P, DO, MHc], BF16)
    bk2 = din("bk2", [P, MOc], F32)
    wv2 = din("wv2", [P, DO, MHc], BF16)
    bv2 = din("bv2", [1, MHc], BF16)
    wo2p = din("wo2p", [P, DO, DO, P], BF16)
    bo2 = din("bo2", [P, DO], F32)
    w1p = din("w1p", [P, FO, DO, P], BF16)
    b1 = din("b1", [P, FO], F32)
    w2p = din("w2p", [P, DO, FO, P], BF16)
    b2 = din("b2", [P, DO], F32)
    g1 = din("g1", [P, DO], F32)
    c1 = din("c1", [P, DO], F32)
    g2 = din("g2", [P, DO], F32)
    c2 = din("c2", [P, DO], F32)
    g3 = din("g3", [P, DO], F32)
    c3 = din("c3", [P, DO], F32)
    strS = din("strS", [P, nss, 1024], BF16)
    strC = din("strC", [P, max(nsc, 1), 2 * SB], BF16)
    y = nc.dram_tensor("y", [DO, P, SB], F32, kind="ExternalOutput").ap()

    with tile.TileContext(nc) as tc:
        res_cm = tc.tile_pool(name="res", bufs=1)
        res = res_cm.__enter__()
        dram_cm = tc.tile_pool(name="dram", bufs=1, space="DRAM")
        dram = dram_cm.__enter__()

        _mm_prev = [None]

        def mm_chain(inst):
            # Total-order the final readers of rotating "mm" PSUM tiles so the
            # scheduler cannot invert drain order against slot capacity.
            if _mm_prev[0] is not None:
                add_dep_helper(inst.ins, _mm_prev[0].ins, sync=True,
                               reason="mm drain order")
            _mm_prev[0] = inst

        psm_cm = tc.tile_pool(name="psm", bufs=2)
        psm = psm_cm.__enter__()
        pln_cm = tc.tile_pool(name="pln", bufs=1)
        pln = pln_cm.__enter__()
        patt_cm = tc.tile_pool(name="patt", bufs=3)
        patt = patt_cm.__enter__()
        phB_cm = tc.tile_pool(name="phB", bufs=1, side="right")
        phB = phB_cm.__enter__()

        # ---- resident small tensors (issued first: they are tiny) --------
        def load_res(ap_in, shape, dt, name):
            t = res.tile(shape, dt, name=name)
            nc.sync.dma_start(t[:], ap_in)
            return t

        bq1t = load_res(bq1, [P, 1], F32, "bq1t")
        bk1t = load_res(bk1, [P, 1], F32, "bk1t")
        bv1t = load_res(bv1, [1, MHs], BF16, "bv1t")
        bo1t = load_res(bo1, [P, DO], F32, "bo1t")
        bq2t = load_res(bq2, [P, DO], F32, "bq2t")
        bk2t = load_res(bk2, [P, MOc], F32, "bk2t")
        bv2t = load_res(bv2, [1, MHc], BF16, "bv2t")
        bo2t = load_res(bo2, [P, DO], F32, "bo2t")
        b1t = load_res(b1, [P, FO], F32, "b1t")
        b2t = load_res(b2, [P, DO], F32, "b2t")
        if not ln_identity:
            g1t = load_res(g1, [P, DO], F32, "g1t")
            c1t = load_res(c1, [P, DO], F32, "c1t")
            g2t = load_res(g2, [P, DO], F32, "g2t")
            c2t = load_res(c2, [P, DO], F32, "c2t")
            g3t = load_res(g3, [P, DO], F32, "g3t")
            c3t = load_res(c3, [P, DO], F32, "c3t")
        else:
            g1t = c1t = g2t = c2t = g3t = c3t = None
        strSt = load_res(strS, [P, nss, 1024], BF16, "strSt")
        strCt = load_res(strC, [P, max(nsc, 1), 2 * SB], BF16, "strCt") \
            if nsc > 0 else None
        onesb = res.tile([1, P], BF16, name="onesb")
        nc.vector.memset(onesb[:], 1.0)
        ones128 = res.tile([P, P], BF16, name="ones128")
        nc.vector.memset(ones128[:], 1.0)
        shiftc = res.tile([P, 1], F32, name="shiftc")
        nc.vector.memset(shiftc[:], EXP_SHIFT)
        epsc = res.tile([P, 1], F32, name="epsc")
        nc.vector.memset(epsc[:], EPS)

        # ---- input weights first, then x/enc per 512-token block ---------
        pio_e_cm = tc.tile_pool(name="pio_e", bufs=1)
        pio_e = pio_e_cm.__enter__()
        pio_x_cm = tc.tile_pool(name="pio_x", bufs=1)
        pio_x = pio_x_cm.__enter__()
        WQ1 = pio_x.tile([P, DO, MHs], BF16, name="WQ1")
        nc.sync.dma_start(WQ1[:], wq1)
        WK1 = pio_x.tile([P, DO, MHs], BF16, name="WK1")
        nc.sync.dma_start(WK1[:], wk1)
        WV1 = pio_x.tile([P, DO, MHs], BF16, name="WV1")
        nc.sync.dma_start(WV1[:], wv1)
        WK2 = pio_e.tile([P, DO, MHc], BF16, name="WK2")
        nc.sync.dma_start(WK2[:], wk2)
        WV2 = pio_e.tile([P, DO, MHc], BF16, name="WV2")
        nc.sync.dma_start(WV2[:], wv2)

        # ================= segment A: all QKV-style projections ===========
        ppA_cm = tc.tile_pool(name="ppA", bufs=3, space="PSUM")
        ppA = ppA_cm.__enter__()

        def qk_drain(dst, ps, bias_t):
            # PSUM -> SBUF bf16; bias add on ACT unless statically zero.
            if qk_bias_zero:
                mm_chain(nc.vector.tensor_copy(dst, ps))
            else:
                mm_chain(nc.scalar.activation(dst, ps, AF.Identity,
                                              bias=bias_t))

        def qk_proj(out, wt, bias_t, src, mo_n, seq):
            for m in range(mo_n):
                for n in range(seq // 512):
                    ps = ppA.tile([P, 512], F32, tag="mm", name="ps_qk")
                    for o in range(DO):
                        nc.tensor.matmul(
                            ps[:], wt[:, o, m * P:(m + 1) * P],
                            src[:, o, n * 512:(n + 1) * 512],
                            start=(o == 0), stop=(o == DO - 1))
                    qk_drain(out[:, m, n * 512:(n + 1) * 512], ps[:],
                             bias_t[:, m:m + 1] if bias_t is not None else None)

        def v_proj(out, wt, bias_row, src, hl, seq):
            # out: [P(tok), seq//P, hl, 2*DK]; cols DK: stay for the ones blk
            mh = hl * DK
            for ms in range(seq // P):
                ps = ppA.tile([P, 512], F32, tag="mm", name="ps_v")
                for o in range(DO):
                    nc.tensor.matmul(
                        ps[:, 0:mh], src[:, o, ms * P:(ms + 1) * P],
                        wt[:, o, :], start=(o == 0),
                        stop=(v_bias_zero and o == DO - 1))
                if not v_bias_zero:
                    nc.tensor.matmul(ps[:, 0:mh], onesb[0:1, :],
                                     bias_row[0:1, :], start=False, stop=True)
                mm_chain(nc.vector.tensor_copy(
                    out[:, ms, :, 0:DK],
                    ps[:, 0:mh].rearrange("p (h d) -> p h d", h=hl)))

        # ---- self QKV projections; KT1z[z] is head z's K zero-padded to
        # 128 partitions so scores run 128-contract (no row-mode switches) --
        QT1 = phB.tile([P, 1, TQ], BF16, name="QT1")
        KT1z = phB.tile([P, 2, TQ], BF16, name="KT1z")
        V1 = phB.tile([P, NKs, HLs, 2 * DK], BF16, name="V1")
        nc.gpsimd.memset(KT1z[DK:2 * DK, 0, :], 0.0)
        nc.gpsimd.memset(KT1z[0:DK, 1, :], 0.0)
        nc.gpsimd.memset(V1[:, :, :, DK:2 * DK], 1.0)
        # Q, K and V of each 512-token block are consumed back-to-back from
        # a rotating 3-deep block buffer: the PE keeps up with (not ahead
        # of) the per-block xT DMAs and only ~24KB/partition is staged.
        for n in range(TQ // 512):
            cols = slice(n * 512, (n + 1) * 512)
            XTb = pio_x.tile([P, DO, 512], BF16, tag="xtb", name="XTb",
                             bufs=3)
            nc.sync.dma_start(XTb[:], xT[:, :, cols])
            ps = ppA.tile([P, 512], F32, tag="mm", name="ps_q1")
            for o in range(DO):
                nc.tensor.matmul(ps[:], WQ1[:, o, :], XTb[:, o, :],
                                 start=(o == 0), stop=(o == DO - 1))
            qk_drain(QT1[:, 0, cols], ps[:], bq1t[:, 0:1])
            ps = ppA.tile([P, 512], F32, tag="mm", name="ps_k1")
            for o in range(DO):
                nc.tensor.matmul(ps[:], WK1[:, o, :], XTb[:, o, :],
                                 start=(o == 0), stop=(o == DO - 1))
            qk_drain(KT1z[0:DK, 0, cols], ps[0:DK], bk1t[0:DK, 0:1])
            qk_drain(KT1z[DK:2 * DK, 1, cols], ps[DK:2 * DK],
                     bk1t[DK:2 * DK, 0:1])
            for ms in range(4 * n, 4 * n + 4):
                ps = ppA.tile([P, 512], F32, tag="mm", name="ps_v")
                for o in range(DO):
                    nc.tensor.matmul(
                        ps[:, 0:MHs],
                        XTb[:, o, (ms % 4) * P:(ms % 4 + 1) * P],
                        WV1[:, o, :], start=(o == 0),
                        stop=(v_bias_zero and o == DO - 1))
                if not v_bias_zero:
                    nc.tensor.matmul(ps[:, 0:MHs], onesb[0:1, :],
                                     bv1t[0:1, :], start=False, stop=True)
                mm_chain(nc.vector.tensor_copy(
                    V1[:, ms, :, 0:DK],
                    ps[:, 0:MHs].rearrange("p (h d) -> p h d", h=HLs)))
        pio_x_cm.__exit__(None, None, None)

        # ---- cross K/V projections (head-interleaved; see module doc) ----
        # V is stored per AG-half as [slot, k, 2*DK] with the attv ones
        # block interleaved, so the AllGather ships attv-ready stationary
        # data and every DMA on the path is contiguous (2KB+ packets).
        KT2l = phB.tile([P, MOc, S], BF16, name="KT2l")
        V2lh = [phB.tile([P, 2, NKc, DK], BF16, name=f"V2lh{i}")
                for i in range(2)]
        for n in range(S // 512):
            cols = slice(n * 512, (n + 1) * 512)
            ENCb = pio_e.tile([P, DO, 512], BF16, tag="encb", name="ENCb",
                              bufs=3)
            nc.sync.dma_start(ENCb[:], encT[:, :, cols])
            for m in range(MOc):
                ps = ppA.tile([P, 512], F32, tag="mm", name="ps_k2")
                for o in range(DO):
                    nc.tensor.matmul(ps[:], WK2[:, o, m * P:(m + 1) * P],
                                     ENCb[:, o, :],
                                     start=(o == 0), stop=(o == DO - 1))
                qk_drain(KT2l[:, m, cols], ps[:], bk2t[:, m:m + 1])
            for ms in range(4 * n, 4 * n + 4):
                ps = ppA.tile([P, 512], F32, tag="mm", name="ps_v2")
                for o in range(DO):
                    nc.tensor.matmul(
                        ps[:, 0:MHc],
                        ENCb[:, o, (ms % 4) * P:(ms % 4 + 1) * P],
                        WV2[:, o, :], start=(o == 0),
                        stop=(v_bias_zero and o == DO - 1))
                if not v_bias_zero:
                    nc.tensor.matmul(ps[:, 0:MHc], onesb[0:1, :],
                                     bv2t[0:1, :], start=False, stop=True)
                for i in range(2):
                    mm_chain(nc.vector.tensor_copy(
                        V2lh[i][:, :, ms, :],
                        ps[:, i * P:(i + 1) * P].rearrange(
                            "p (h d) -> p h d", h=2)))
        pio_e_cm.__exit__(None, None, None)
        ppA_cm.__exit__(None, None, None)

        # each AG half: K mo-block (2 local heads) + matching V slots
        KH = S            # K half size per partition
        VH = 2 * NKc * DK # V half size per partition (compact)
        kv_loc = [dram.tile([P, KH + VH], BF16, name=f"kv_loc{i}")
                  for i in range(2)]
        kv_ag = [dram.tile([G, P, KH + VH], BF16, name=f"kv_ag{i}")
                 for i in range(2)]
        nc.sync.dma_start(kv_loc[0][:, 0:KH], KT2l[:, 0, :])
        nc.sync.dma_start(kv_loc[0][:, KH:KH + VH],
                          V2lh[0].rearrange("p s k d -> p (s k d)"))
        # AG1 now; the a2a's and then AG2 follow self-attention.  AG2's
        # input DMAs are also held back until then: the CC scheduler runs
        # whichever collective has ready inputs, and a ready AG2 would
        # block the latency-critical a2as for ~90us.
        nc.gpsimd.collective_compute(
            "AllGather", ALU.bypass, replica_groups=groups4,
            ins=[kv_loc[0].opt()], outs=[kv_ag[0].opt()])

        # cross-attn K/V staging allocated early: the ones-padding memsets
        # must land on the gpsimd queue BEFORE the a2a/AG2 collectives.
        # Cross attention runs in 4 head-quarters (4 heads each) to halve
        # the Va staging footprint; quarters 0,1 come from AG half 0.
        HQ = 4
        pva_cm = tc.tile_pool(name="pva", bufs=1)
        pva = pva_cm.__enter__()
        pkt_cm = tc.tile_pool(name="pkt", bufs=1)
        pkt = pkt_cm.__enter__()
        ktpz = [pkt.tile([P, 2, S], BF16, name=f"ktpz{i}") for i in range(2)]
        for i in range(2):
            nc.gpsimd.memset(ktpz[i][DK:2 * DK, 0, :], 0.0)
            nc.gpsimd.memset(ktpz[i][0:DK, 1, :], 0.0)
        VaB = [pva.tile([P, HQ, NKc, 2 * DK], BF16, name=f"VaB{i}")
               for i in range(2)]
        for i in range(2):
            nc.gpsimd.memset(VaB[i][:, :, :, DK:2 * DK], 1.0)


        # post-attention working set (DMAs run under attention) ------------
        pO1_cm = tc.tile_pool(name="pO1", bufs=1)
        pO1 = pO1_cm.__enter__()
        xckt = pO1.tile([P, DO, SB], F32, name="xckt")
        nc.sync.dma_start(xckt[:], xck)

        # ================= segment B: self attention ======================
        ppB_cm = tc.tile_pool(name="ppB", bufs=2, space="PSUM")
        ppB = ppB_cm.__enter__()

        def attention_pair(qt_m, KT, QT, vt, he, ho, qb, cls_row, strips,
                           Xatt, xm, nk):
            """One (head-pair, q-block).  Even head lives on SBUF partitions
            0-63, odd head on 64-127.  Vt[..., DK:2DK] is an all-ones block,
            so each attV matmul emits 64 broadcast copies of the softmax
            denominator on partitions 64-127; normalization is pure DVE.
            Both heads' scores land in one [128,1024] PSUM tile -> a single
            exp ACT instruction; scores run PIPE blocks ahead of attV."""
            allowed = [kb for kb in range(nk) if cls_row[kb] != SKIP]
            n = len(allowed)
            pa_e = ppB.tile([P, 512], F32, tag="pa", name="pa_e")
            pa_o = ppB.tile([P, 512], F32, tag="pa", name="pa_o")
            Es = {}

            def do_scores(j):
                kb = allowed[j]
                ps2 = ppB.tile([P, 1024], F32, tag="sc", name="ps_s", bufs=3)
                for half in (0, 1):
                    nc.tensor.matmul(
                        ps2[:, half * 512:half * 512 + 512],
                        KT[:, half, kb * P:(kb + 1) * P],
                        QT[:, qt_m, qb * 512:(qb + 1) * 512],
                        start=True, stop=True)
                if cls_row[kb] >= 0:
                    nc.vector.tensor_tensor(
                        ps2[:], ps2[:], strips[:, cls_row[kb], :], ALU.add)
                E = patt.tile([P, 1024], BF16, tag="E", name="E", bufs=4)
                mm_chain(nc.scalar.activation(E[:], ps2[:], AF.Exp,
                                              bias=shiftc[:, 0:1]))
                Es[j] = E

            def do_attv(j):
                kb = allowed[j]
                E = Es.pop(j)
                nc.tensor.matmul(pa_e[:], vt(kb, he), E[:, 0:512],
                                 start=(j == 0), stop=(j == n - 1))
                nc.tensor.matmul(pa_o[:], vt(kb, ho), E[:, 512:1024],
                                 start=(j == 0), stop=(j == n - 1))

            for j in range(n):
                do_scores(j)
                if j >= PIPE:
                    do_attv(j - PIPE)
            for j in range(max(0, n - PIPE), n):
                do_attv(j)

            # denominators: approx-reciprocal straight out of PSUM (fp32)
            rec = psm.tile([P, 512], F32, tag="rec", name="rec")
            if USE_FAST_RECIP:
                den = psm.tile([P, 512], F32, tag="den", name="den")
                nc.vector.tensor_copy(den[0:DK, :], pa_e[DK:2 * DK, :])
                nc.vector.tensor_copy(den[DK:2 * DK, :], pa_o[DK:2 * DK, :])
                nc.vector.reciprocal_approx_fast(rec[:, :], den[:, :])
            else:
                den = psm.tile([P, 512], BF16, tag="den", name="den")
                nc.vector.tensor_copy(den[0:DK, :], pa_e[DK:2 * DK, :])
                nc.vector.tensor_copy(den[DK:2 * DK, :], pa_o[DK:2 * DK, :])
                with nc.allow_low_precision(reason="softmax denom recip"):
                    nc.vector.reciprocal(rec[:, :], den[:, :])
            nc.vector.tensor_tensor(
                Xatt[0:DK, xm, :], pa_e[0:DK, :], rec[0:DK, :], ALU.mult)
            nc.vector.tensor_tensor(
                Xatt[DK:2 * DK, xm, :], pa_o[0:DK, :], rec[DK:2 * DK, :],
                ALU.mult)

        # ---- self attention + AllToAll context exchange ------------------
        HB = SB // 2
        a2a_inA = dram.tile([cfg.n_cores, P, HB], BF16, name="a2a_inA")
        a2a_inB = dram.tile([cfg.n_cores, P, HB], BF16, name="a2a_inB")

        a2a_in_insts = []

        def self_qb(qb):
            xst = pln.tile([P, 1, SB], BF16, tag="xst", name="xst", bufs=8)
            attention_pair(0, KT1z, QT1,
                           lambda kb, h: V1[:, kb, h, :], 0, 1, qb,
                           self_cls[qb], strSt, xst, 0, NKs)
            a2a_in_insts.append(nc.sync.dma_start(a2a_inA[qb],
                                                  xst[:, 0, 0:HB]))
            a2a_in_insts.append(nc.sync.dma_start(a2a_inB[qb],
                                                  xst[:, 0, HB:SB]))

        # interleave strip-heavy (small) and strip-light (large) q-blocks so
        # neither the DVE mask work nor the PE matmul work piles up.
        sz = lambda q: sum(1 for v in self_cls[q] if v != SKIP)
        asc = sorted(range(NQ), key=sz)
        qb_order = []
        for i in range(NQ // 2):
            qb_order += [asc[i], asc[NQ - 1 - i]]
        for qb in qb_order:
            self_qb(qb)
        ppB_cm.__exit__(None, None, None)

        a2a_outA = dram.tile([cfg.n_cores, P, HB], BF16, name="a2a_outA")
        a2a_outB = dram.tile([cfg.n_cores, P, HB], BF16, name="a2a_outB")
        a2aA_coll = nc.gpsimd.collective_compute(
            "AllToAll", ALU.bypass, replica_groups=groups8,
            ins=[a2a_inA.opt()], outs=[a2a_outA.opt()])
        a2aB_coll = nc.gpsimd.collective_compute(
            "AllToAll", ALU.bypass, replica_groups=groups8,
            ins=[a2a_inB.opt()], outs=[a2a_outB.opt()])
        # AG2's input write is chained behind the final a2a input so the
        # CC cannot start the (long) AG2 before the latency-critical a2as:
        # the CC scheduler runs whatever has ready inputs first.
        kvd1 = nc.sync.dma_start(kv_loc[1][:, 0:KH], KT2l[:, 1, :])
        kvd2 = nc.sync.dma_start(kv_loc[1][:, KH:KH + VH],
                                 V2lh[1].rearrange("p s k d -> p (s k d)"))
        for kvd in (kvd1, kvd2):
            add_dep_helper(kvd.ins, a2a_in_insts[-1].ins, sync=True,
                           reason="hold AG2 input behind a2a inputs")
        nc.gpsimd.collective_compute(
            "AllGather", ALU.bypass, replica_groups=groups4,
            ins=[kv_loc[1].opt()], outs=[kv_ag[1].opt()])
        phB_cm.__exit__(None, None, None)
        XA = pO1.tile([P, DO, SB], BF16, name="XA")
        nc.sync.dma_start(XA[:, :, 0:HB],
                          a2a_outA.rearrange("j p h -> p j h"))
        nc.sync.dma_start(XA[:, :, HB:SB],
                          a2a_outB.rearrange("j p h -> p j h"))

        # ================= segment C: O1 + LN1 + Q2 =======================
        ppC_cm = tc.tile_pool(name="ppC", bufs=2, space="PSUM")
        ppC = ppC_cm.__enter__()

        # ---- LN helpers (stats pre-broadcast via all-ones stationary;
        # per-m stat matmuls are emitted lag-one inside the producer loops
        # so the PE never drains between a projection and its LN) ----------
        def ln_stats_begin(pp):
            psA = pp.tile([P, 512], F32, tag="stat", name="psA", bufs=2)
            psB = pp.tile([P, 512], F32, tag="stat", name="psB", bufs=2)
            return psA, psB

        def ln_stat_m(st, S_sb, m, W):
            psA, psB = st
            sbf = pln.tile([P, W], BF16, tag="sbf", name="sbf", bufs=2)
            sq = pln.tile([P, W], BF16, tag="sq", name="sq", bufs=2)
            nc.vector.tensor_copy(sbf[:], S_sb[:, m])
            nc.vector.tensor_mul(sq[:], S_sb[:, m], S_sb[:, m])
            nc.tensor.matmul(psA[:, 0:W], ones128[:], sbf[:],
                             start=(m == 0), stop=(m == DO - 1))
            nc.tensor.matmul(psB[:, 0:W], ones128[:], sq[:],
                             start=(m == 0), stop=(m == DO - 1))

        def ln_finish(st, S_sb, gt, ct, out_f32, out_bf16, W,
                      out_cb=None):
            psA, psB = st
            mu = psm.tile([P, 512], F32, tag="stat", name="mu", bufs=6)
            nc.vector.tensor_scalar_mul(mu[:, 0:W], psA[:, 0:W], 1.0 / D)
            e2 = psm.tile([P, 512], F32, tag="stat", name="e2", bufs=6)
            nc.vector.tensor_scalar_mul(e2[:, 0:W], psB[:, 0:W], 1.0 / D)
            var = psm.tile([P, 512], F32, tag="stat", name="var", bufs=6)
            nc.vector.tensor_mul(var[:, 0:W], mu[:, 0:W], mu[:, 0:W])
            nc.vector.tensor_sub(var[:, 0:W], e2[:, 0:W], var[:, 0:W])
            # rstd = exp(-0.5 * ln(var + eps)) -- stays in the exp table set
            lnv = psm.tile([P, 512], F32, tag="stat", name="lnv", bufs=6)
            nc.scalar.activation(lnv[:, 0:W], var[:, 0:W], AF.Ln,
                                 bias=epsc[:, 0:1])
            rstd = psm.tile([P, 512], F32, tag="stat", name="rstd", bufs=6)
            nc.scalar.activation(rstd[:, 0:W], lnv[:, 0:W], AF.Exp,
                                 scale=-0.5)
            mr = psm.tile([P, 512], F32, tag="stat", name="mr", bufs=6)
            nc.vector.tensor_mul(mr[:, 0:W], mu[:, 0:W], rstd[:, 0:W])
            for m in range(DO):
                if ln_identity:
                    t2 = pln.tile([P, W], F32, tag="t2", name="t2", bufs=2)
                    nc.vector.tensor_mul(t2[:], S_sb[:, m], rstd[:, 0:W])
                    nc.vector.tensor_sub(out_f32[:, m], t2[:], mr[:, 0:W])
                else:
                    t2 = pln.tile([P, W], F32, tag="t2", name="t2", bufs=2)
                    nc.vector.tensor_mul(t2[:], S_sb[:, m], rstd[:, 0:W])
                    nc.vector.tensor_sub(t2[:], t2[:], mr[:, 0:W])
                    nc.scalar.activation(out_f32[:, m], t2[:], AF.Identity,
                                         bias=ct[:, m:m + 1],
                                         scale=gt[:, m:m + 1])
                if out_bf16 is not None:
                    nc.vector.tensor_copy(out_bf16[:, m], out_f32[:, m])
                if out_cb is not None:
                    out_cb(m)

        pw_cm = tc.tile_pool(name="pw", bufs=4, side="right")
        pw = pw_cm.__enter__()
        pC_cm = tc.tile_pool(name="pC", bufs=1, side="right")
        pC = pC_cm.__enter__()

        # ---- O1 projection (full D) + residual + LN1 ---------------------
        S1 = pO1.tile([P, DO, SB], F32, name="S1")
        st1 = ln_stats_begin(ppC)
        for h in range(2):
            cr = slice(h * HB, (h + 1) * HB)
            for m in range(DO):
                wo1t = pw.tile([P, DO, P], BF16, tag="wsm", name="wo1t")
                nc.sync.dma_start(wo1t[:], wo1p[:, m])
                ps = ppC.tile([P, 512], F32, tag="mm", name="ps_o1", bufs=3)
                for j in range(DO):
                    nc.tensor.matmul(
                        ps[:, 0:HB], wo1t[:, j, :],
                        XA[:, j, cr], start=(j == 0), stop=(j == DO - 1))
                mm_chain(nc.vector.scalar_tensor_tensor(
                    S1[:, m, cr], ps[:, 0:HB], bo1t[:, m:m + 1],
                    xckt[:, m, cr], ALU.add, ALU.add))
                if h == 1 and m > 0:
                    ln_stat_m(st1, S1, m - 1, SB)
        ln_stat_m(st1, S1, DO - 1, SB)
        X2f = pC.tile([P, DO, SB], F32, name="X2f")
        X2b = pC.tile([P, DO, SB], BF16, name="X2b")
        ln_finish(st1, S1, g1t, c1t, X2f, X2b, SB)
        if debug_stage == 1:
            for m in range(DO):
                nc.sync.dma_start(y[m], X2f[:, m])
        pO1_cm.__exit__(None, None, None)

        # ---- cross-attn Q projection (token-sharded, all heads) ----------
        Q2T = pC.tile([P, DO, SB], BF16, name="Q2T")
        for m in range(DO):
            wq2t = pw.tile([P, DO, P], BF16, tag="wsm", name="wq2t")
            nc.sync.dma_start(wq2t[:], wq2p[:, m])
            ps = ppC.tile([P, 512], F32, tag="mm", name="ps_q2", bufs=3)
            for o in range(DO):
                nc.tensor.matmul(ps[:, 0:SB], wq2t[:, o, :], X2b[:, o, :],
                                 start=(o == 0), stop=(o == DO - 1))
            qk_drain(Q2T[:, m, :], ps[:, 0:SB], bq2t[:, m:m + 1])
        ppC_cm.__exit__(None, None, None)

        # ================= segment D: cross attention =====================
        ppD_cm = tc.tile_pool(name="ppD", bufs=2, space="PSUM")
        ppD = ppD_cm.__enter__()
        # reuse attention_pair's pool variable
        ppB = ppD

        # head h lives on AG rank h%G, local slot (h//G)%2 of half h//(2*G)
        Xatt2 = pC.tile([P, DO, SB], BF16, name="Xatt2")
        cross_cls = [kb if nsc > 0 else FULL for kb in range(NKc)]
        VSL = NKc * DK  # one slot's V elements per partition
        for q in range(H // HQ):
            Va = VaB[q % 2]
            half = q // 2
            h0 = q * HQ
            for h in range(h0, h0 + HQ):
                r, s = h % G, (h // G) % 2
                Vc = pva.tile([P, NKc * DK], BF16, tag="vac", name="Vc",
                              bufs=2)
                vdma = nc.sync.dma_start(
                    Vc[:],
                    kv_ag[half][r, :, KH + s * VSL:KH + (s + 1) * VSL])
                if half == 0:
                    add_dep_helper(vdma.ins, a2aB_coll.ins, sync=True,
                                   reason="Va loads after the a2as")
                nc.vector.tensor_copy(
                    Va[:, h - h0, :, 0:DK],
                    Vc[:].rearrange("p (k d) -> p k d", k=NKc))
            for hp in range(h0 // 2, (h0 + HQ) // 2):
                kz = ktpz[hp % 2]
                for z, h in ((0, 2 * hp), (1, 2 * hp + 1)):
                    r, s = h % G, (h // G) % 2
                    kdma = nc.sync.dma_start(
                        kz[z * DK:(z + 1) * DK, z, :],
                        kv_ag[half][r, s * DK:(s + 1) * DK, 0:KH])
                    if half == 0:
                        add_dep_helper(kdma.ins, a2aB_coll.ins, sync=True,
                                       reason="kz loads after the a2as")
                attention_pair(hp, kz, Q2T,
                               lambda kb, h: Va[:, h, kb, :],
                               2 * hp - h0, 2 * hp + 1 - h0, 0, cross_cls,
                               strCt, Xatt2, hp, NKc)
        pkt_cm.__exit__(None, None, None)
        pva_cm.__exit__(None, None, None)
        ppD_cm.__exit__(None, None, None)

        # ================= segment E: O2 + LN2 + FFN + LN3 ================
        ppE_cm = tc.tile_pool(name="ppE", bufs=2, space="PSUM")
        ppE = ppE_cm.__enter__()
        pff_cm = tc.tile_pool(name="pff", bufs=1)
        pff = pff_cm.__enter__()
        FH = FO // 2
        # prefetch the first fc1 weight half while cross O-proj runs; the
        # second half streams through pw during the first half's matmuls
        W1h0 = pff.tile([P, FH, DO, P], BF16, name="W1h0")
        for mf in range(FH):
            wdma = nc.sync.dma_start(W1h0[:, mf], w1p[:, mf])
            add_dep_helper(wdma.ins, a2aB_coll.ins, sync=True,
                           reason="fc1 prefetch after the a2as")

        # ---- cross O-projection + residual (in-place over X2f) + LN2 ----
        st2 = ln_stats_begin(ppE)
        for m in range(DO):
            wo2t = pw.tile([P, DO, P], BF16, tag="wsm", name="wo2t")
            nc.sync.dma_start(wo2t[:], wo2p[:, m])
            ps = ppE.tile([P, 512], F32, tag="mm", name="ps_o2", bufs=3)
            for o in range(DO):
                nc.tensor.matmul(ps[:, 0:SB], wo2t[:, o, :], Xatt2[:, o, :],
                                 start=(o == 0), stop=(o == DO - 1))
            mm_chain(nc.vector.scalar_tensor_tensor(
                X2f[:, m], ps[:, 0:SB], bo2t[:, m:m + 1], X2f[:, m],
                ALU.add, ALU.add))
            if m > 0:
                ln_stat_m(st2, X2f, m - 1, SB)
        ln_stat_m(st2, X2f, DO - 1, SB)
        X4f = pff.tile([P, DO, SB], F32, name="X4f")
        X4b = pff.tile([P, DO, SB], BF16, name="X4b")
        ln_finish(st2, X2f, g2t, c2t, X4f, X4b, SB)
        if debug_stage == 2:
            for m in range(DO):
                nc.sync.dma_start(y[m], X4f[:, m])
        pC_cm.__exit__(None, None, None)

        # ---- FFN (two FO-halves; accumulate into X4f in place) ----------
        st3 = ln_stats_begin(ppE)
        for half in range(2):
            f0 = half * FH
            Ht = pff.tile([P, FH, SB], BF16, tag="Ht", name="Ht", bufs=2)
            for mf in range(FH):
                if half == 0:
                    w1t = W1h0[:, mf]
                else:
                    w1t = pw.tile([P, DO, P], BF16, tag="wsm", name="w1t")
                    nc.sync.dma_start(w1t[:], w1p[:, f0 + mf])
                ps = ppE.tile([P, 512], F32, tag="mm", name="ps_f1", bufs=3)
                for o in range(DO):
                    nc.tensor.matmul(ps[:, 0:SB], w1t[:, o, :],
                                     X4b[:, o, :],
                                     start=(o == 0), stop=(o == DO - 1))
                mm_chain(nc.scalar.activation(
                    Ht[:, mf, :], ps[:, 0:SB], AF.Relu,
                    bias=b1t[:, f0 + mf:f0 + mf + 1]))
            for m in range(DO):
                w2t = pw.tile([P, FH, P], BF16, tag="w2", name="w2t", bufs=3)
                nc.sync.dma_start(w2t[:], w2p[:, m, f0:f0 + FH, :])
                ps = ppE.tile([P, 512], F32, tag="mm", name="ps_f2", bufs=3)
                for of in range(FH):
                    nc.tensor.matmul(ps[:, 0:SB], w2t[:, of, :], Ht[:, of, :],
                                     start=(of == 0), stop=(of == FH - 1))
                if half == 0:
                    mm_chain(nc.vector.scalar_tensor_tensor(
                        X4f[:, m], ps[:, 0:SB], b2t[:, m:m + 1], X4f[:, m],
                        ALU.add, ALU.add))
                else:
                    mm_chain(nc.vector.tensor_add(
                        X4f[:, m], X4f[:, m], ps[:, 0:SB]))
                    if m > 0:
                        ln_stat_m(st3, X4f, m - 1, SB)
        ln_stat_m(st3, X4f, DO - 1, SB)
        ln_finish(st3, X4f, g3t, c3t, X4f, None, SB,
                  out_cb=(lambda m: nc.sync.dma_start(y[m], X4f[:, m]))
                  if debug_stage == 0 else None)

        pff_cm.__exit__(None, None, None)
        ppE_cm.__exit__(None, None, None)
        pw_cm.__exit__(None, None, None)
        patt_cm.__exit__(None, None, None)
        pln_cm.__exit__(None, None, None)
        pC_cm.__exit__(None, None, None)
        psm_cm.__exit__(None, None, None)
        dram_cm.__exit__(None, None, None)
        res_cm.__exit__(None, None, None)

    nc.compile()
    return nc


# ---------------------------------------------------------------------------
# host side
# ---------------------------------------------------------------------------

def _pack_ko(a):
    """[K, F] -> [128, K//128, F] (contract dim on partitions)."""
    K, F = a.shape
    return np.ascontiguousarray(a.reshape(K // P, P, F).transpose(1, 0, 2))


def _pack_vec(v, n):
    return np.ascontiguousarray(np.asarray(v, np.float32).reshape(n, P).T)


def classify_self(mask, NQ, NK):
    """mask [S, S] bool (q, k). Returns cls [NQ][NK] and strips [128, nss, 512]."""
    cls = [[FULL] * NK for _ in range(NQ)]
    strips = []
    keys = {}
    for qb in range(NQ):
        for kb in range(NK):
            blk = mask[qb * 512:(qb + 1) * 512, kb * P:(kb + 1) * P]
            if blk.all():
                cls[qb][kb] = FULL
            elif not blk.any():
                cls[qb][kb] = SKIP
            else:
                key = blk.tobytes()
                if key not in keys:
                    keys[key] = len(strips)
                    strips.append(np.where(blk.T, np.float32(0),
                                           np.float32(NEG)))
                cls[qb][kb] = keys[key]
    if strips:
        arr = np.stack(strips, 0).transpose(1, 0, 2)
    else:
        arr = np.zeros((P, 1, 512), np.float32)
    arr = np.concatenate([arr, arr], axis=2)  # same strip for both heads
    return cls, np.ascontiguousarray(arr).astype(bfloat16)


_CACHE = {}
DEBUG_STAGE = 0


def kernel(**inputs):
    cfg = Cfg(S=2048, D=1024, H=16, FF=4096, TP=4, B=2)
    return _run(cfg, inputs)


def _run(cfg, inputs, trace=False):
    S, D, G, B, SB, DO = cfg.S, cfg.D, cfg.G, cfg.B, cfg.SB, cfg.DO
    MHs, MHc, MOc, NQ, NKs, NKc = (cfg.MHs, cfg.MHc, cfg.MOc, cfg.NQ,
                                   cfg.NKs, cfg.NKc)
    f32 = np.float32
    bf = bfloat16
    tgt = np.asarray(inputs["tgt_mask"])[0, 0] != 0
    src = np.asarray(inputs["src_mask"])[0, 0] != 0

    # per-batch causal classification, composed block-diagonally over B
    clsb, strS = classify_self(tgt, S // 512, S // P)
    nqb, nkb = S // 512, S // P
    self_cls = [[SKIP] * NKs for _ in range(NQ)]
    for qb in range(NQ):
        for kb in range(NKs):
            if qb // nqb == kb // nkb:
                self_cls[qb][kb] = clsb[qb % nqb][kb % nkb]
    nsc = 0 if src.all() else NKc

    v_bias_zero = (not np.asarray(inputs["m1_bv"]).any()) and \
        (not np.asarray(inputs["m2_bv"]).any())
    qk_bias_zero = not any(np.asarray(inputs[k]).any() for k in
                           ("m1_bq", "m1_bk", "m2_bq", "m2_bk"))
    ln_identity = all(
        (np.asarray(inputs[g]) == 1).all() and
        (not np.asarray(inputs[c]).any())
        for g, c in (("ln1_g", "ln1_b"), ("ln2_g", "ln2_b"),
                     ("ln3_g", "ln3_b")))
    key = (cfg.S, cfg.D, cfg.H, cfg.FF, cfg.G, cfg.B,
           tuple(map(tuple, self_cls)), nsc, v_bias_zero, qk_bias_zero,
           ln_identity, DEBUG_STAGE, USE_FAST_RECIP)
    if key not in _CACHE:
        _CACHE[key] = build_program(cfg, self_cls, nsc, v_bias_zero,
                                    qk_bias_zero, ln_identity, DEBUG_STAGE)
    nc = _CACHE[key]

    x = np.asarray(inputs["x"], f32)
    enc = np.asarray(inputs["enc_out"], f32)
    w1 = np.asarray(inputs["ff_w1"], f32)
    w2 = np.asarray(inputs["ff_w2"], f32)
    wq2 = np.asarray(inputs["m2_wq"], f32)
    wo2 = np.asarray(inputs["m2_wo"], f32)

    # xT: both batches concatenated on the token axis (batch-major)
    xT_full = np.concatenate([x[0], x[1]], axis=0).T  # [D, TQ]
    xT_pack = np.ascontiguousarray(
        xT_full.reshape(DO, P, cfg.TQ).transpose(1, 0, 2)).astype(bf)

    shared = {
        "xT": xT_pack,
        "wo1p": np.ascontiguousarray(
            np.asarray(inputs["m1_wo"], f32).reshape(
                DO, P, DO, P).transpose(1, 2, 0, 3)).astype(bf),
        "wq2p": np.ascontiguousarray(
            wq2.reshape(DO, P, DO, P).transpose(1, 2, 0, 3)).astype(bf),
        "bq2": _pack_vec(inputs["m2_bq"], DO),
        "wo2p": np.ascontiguousarray(
            wo2.reshape(DO, P, DO, P).transpose(1, 2, 0, 3)).astype(bf),
        "bo2": _pack_vec(inputs["m2_bo"], DO),
        "bo1": _pack_vec(inputs["m1_bo"], DO),
        "w1p": np.ascontiguousarray(
            w1.reshape(DO, P, cfg.FO, P).transpose(1, 2, 0, 3)).astype(bf),
        "b1": _pack_vec(inputs["ff_b1"], cfg.FO),
        "w2p": np.ascontiguousarray(
            w2.reshape(cfg.FO, P, DO, P).transpose(1, 2, 0, 3)).astype(bf),
        "b2": _pack_vec(inputs["ff_b2"], DO),
        "g1": _pack_vec(inputs["ln1_g"], DO),
        "c1": _pack_vec(inputs["ln1_b"], DO),
        "g2": _pack_vec(inputs["ln2_g"], DO),
        "c2": _pack_vec(inputs["ln2_b"], DO),
        "g3": _pack_vec(inputs["ln3_g"], DO),
        "c3": _pack_vec(inputs["ln3_b"], DO),
        "strS": strS,
    }

    in_maps = []
    for c in range(cfg.n_cores):
        b, r = divmod(c, G)
        xTb = np.ascontiguousarray(x[b].T)
        encTb = np.ascontiguousarray(enc[b].T)
        m = dict(shared)
        m["xck"] = np.ascontiguousarray(
            xTb[:, r * SB:(r + 1) * SB].reshape(DO, P, SB).transpose(1, 0, 2))
        m["encT"] = np.ascontiguousarray(
            encTb.reshape(DO, P, S).transpose(1, 0, 2)).astype(bf)
        # self-attn: 2 heads per core (TP=8 over heads)
        hs = slice(c * MHs, (c + 1) * MHs)
        m["wq1"] = _pack_ko(np.asarray(inputs["m1_wq"], f32)[:, hs]).astype(bf)
        m["wk1"] = _pack_ko(np.asarray(inputs["m1_wk"], f32)[:, hs]).astype(bf)
        m["wv1"] = _pack_ko(np.asarray(inputs["m1_wv"], f32)[:, hs]).astype(bf)
        m["bq1"] = _pack_vec(np.asarray(inputs["m1_bq"], f32)[hs], 1)
        m["bk1"] = _pack_vec(np.asarray(inputs["m1_bk"], f32)[hs], 1)
        m["bv1"] = np.asarray(inputs["m1_bv"], f32)[hs].reshape(1, MHs).astype(bf)
        # cross-attn K/V: interleaved heads {r, r+G, r+2G, r+3G} per rank,
        # packed in slot order so AG half i carries slots 2i, 2i+1
        DKc = cfg.DK
        heads_r = [r + G * s for s in range(cfg.HLc)]
        hc_cols = np.concatenate(
            [np.arange(h * DKc, (h + 1) * DKc) for h in heads_r])
        m["wk2"] = _pack_ko(
            np.asarray(inputs["m2_wk"], f32)[:, hc_cols]).astype(bf)
        m["wv2"] = _pack_ko(
            np.asarray(inputs["m2_wv"], f32)[:, hc_cols]).astype(bf)
        m["bk2"] = _pack_vec(np.asarray(inputs["m2_bk"], f32)[hc_cols], MOc)
        m["bv2"] = np.asarray(inputs["m2_bv"], f32)[hc_cols].reshape(
            1, MHc).astype(bf)
        if nsc > 0:
            blks = []
            for kb in range(NKc):
                blk = src[r * SB:(r + 1) * SB, kb * P:(kb + 1) * P]
                blks.append(np.where(blk.T, f32(0), f32(NEG)))
            arrc = np.stack(blks, 0).transpose(1, 0, 2)
            arrc = np.concatenate([arrc, arrc], axis=2)
            m["strC"] = np.ascontiguousarray(arrc).astype(bf)
        else:
            m["strC"] = np.zeros((P, 1, 2 * SB), bf)
        in_maps.append(m)

    res = bass_utils.run_bass_kernel_spmd(
        nc, in_maps, core_ids=list(range(cfg.n_cores)), trace=trace)

    out = np.empty((B, S, D), f32)
    for c in range(cfg.n_cores):
        b, r = divmod(c, G)
        yv = res.results[c]["y"]
        out[b, r * SB:(r + 1) * SB, :] = yv.transpose(2, 0, 1).reshape(SB, D)
    if trace:
        return out, res
    return out
